# revision 16
# baseline (speedup 1.0000x reference)
"""Trainium2 Bass kernel for nn_BasicBlock_63874753626191.

TCN encoder (only the last timestep's receptive cone is computed) + AR head
+ two autoregressive LSTM decoders + difference-guided gating MLP.

Distribution: data-parallel over batch across 8 NeuronCores (8 rows each);
weights replicated.

Key algebraic transform: the LSTM feedback input x_t = fcw@h_{t-1} + fcb is
linear in h, so the input term folds into the recurrent weights
(whh_eff = whh + wih@fcw, bias_eff = bih + bhh + wih*fcb).  Both decoders'
step-0 inputs are also linear in `hidden`, handled by alternate step-0
weights.  The per-step work is then exactly one 256->1024 matmul plus the
gate nonlinearities, laid out feature-on-partition.
"""

import numpy as np

B, T, F = 64, 512, 16
C = 256
H = 96
K = 4
NCORES = 8
BL = B // NCORES  # batch rows per core

WIN = 64   # TCN window (last WIN timesteps fully cover the receptive cone)
PAD = 12   # left zero-pad inside the window buffer
WF = WIN + PAD

_FP = np.float32


# ---------------------------------------------------------------------------
# Walrus-compat workarounds for this container's compiler build, which only
# accepts a single sync-wait command per instruction encoding:
#  (1) split the Tile kernel-tail drain into a chain of single-wait drains;
#  (2) post-process the serialized BIR, hoisting extra on_wait entries onto
#      injected EventSemaphore instructions just before the owner.

_EVW_COUNTER = [0]


def _split_sync_waits_json(mod):
    def fix_block(insts):
        out = []
        for ins in insts:
            si = ins.get("sync_info") or {}
            waits = si.get("on_wait") or []
            if len(waits) > 1:
                keep = waits[-1]
                for w in waits[:-1]:
                    _EVW_COUNTER[0] += 1
                    out.append({
                        "debug": ins.get("debug", 0),
                        "engine": ins["engine"],
                        "ins": [],
                        "name": f"evw-{_EVW_COUNTER[0]}",
                        "opcode": "EventSemaphore",
                        "outs": [],
                        "sync_info": {"on_wait": [w], "on_update": []},
                    })
                si = dict(si)
                si["on_wait"] = [keep]
                ins = dict(ins)
                ins["sync_info"] = si
            out.append(ins)
        return out

    def walk(o):
        if isinstance(o, dict):
            for k, v in o.items():
                if k == "instructions" and isinstance(v, list):
                    o[k] = fix_block(v)
                else:
                    walk(v)
        elif isinstance(o, list):
            for v in o:
                walk(v)

    walk(mod)
    return mod


def _install_json_wait_split():
    import json as _json
    import concourse.bass as bass

    if getattr(bass.Bass, "_wait_split_installed", False):
        return
    orig = bass.Bass.to_json_bytes

    def to_json_bytes(self):
        raw = orig(self)
        mod = _json.loads(raw)
        _split_sync_waits_json(mod)
        return _json.dumps(mod).encode()

    bass.Bass.to_json_bytes = to_json_bytes
    bass.Bass._wait_split_installed = True


def _install_tile_drain_fix():
    import concourse.tile as tile
    from concourse.vector_clock import ScopedClock, VectorClock

    _install_json_wait_split()

    def _drain_and_barrier_split(self, tick_clock, wait_clock):
        gc = tick_clock.global_clock
        n = len(gc)
        for p in range(n):
            if gc[p] <= 0:
                continue
            vc = VectorClock([0] * n)
            vc.require_at_least(p, gc[p])
            d = self.nc.sync.drain()
            wait_clock.add_sem_waits(d.ins, ScopedClock({None: vc}))
        self.nc.all_engine_barrier()
        assert self.sems is not None
        popped = self.nc._tile_sem_poison_stack.pop()
        assert popped is self._sem_poison
        self.nc.clear_and_free_semaphores(list(self.sems.allocated().values()))
        self.nc.all_engine_barrier()

    tile.TileContext._drain_and_barrier = _drain_and_barrier_split


# ---------------------------------------------------------------------------
# Host-side weight layout prep (identical on every core).

# Gate chunk permutation: torch order [i f g o] -> device order [i f o g]
# so sigmoid covers chunks 0..5 contiguously and tanh covers 6..7.
_PERM = np.concatenate([np.arange(0, 2 * C), np.arange(3 * C, 4 * C),
                        np.arange(2 * C, 3 * C)])


def _lstm_layout(wih, whh, bih, bhh, fcw, fcb, x0w, x0b):
    """Return (whT, wh0T, bias, bias0) in device layout."""
    wih = wih.astype(_FP)      # [4C, 1]
    whh = whh.astype(_FP)      # [4C, C]
    fcw = fcw.astype(_FP)      # [1, C]
    x0w = x0w.astype(_FP)      # [1, C]
    weff = whh + wih @ fcw                      # steps >= 1
    weff0 = whh + wih @ x0w                     # step 0
    beff = bih + bhh + wih[:, 0] * float(fcb[0])
    beff0 = bih + bhh + wih[:, 0] * float(x0b[0])

    def lay_w(w):
        # [4C, C] -> perm rows -> transpose -> [C, 4C] -> [2, 128, 4C] -> [128, 2, 4C]
        wt = w[_PERM, :].T.astype(_FP)           # [C, 4C]
        return np.ascontiguousarray(
            wt.reshape(2, 128, 4 * C).transpose(1, 0, 2))

    def lay_b(b):
        # [4C] -> perm -> [8, 128] -> [128, 8] -> replicate over batch [128, 8, BL]
        bb = b[_PERM].astype(_FP).reshape(8, 128).T
        return np.ascontiguousarray(
            np.repeat(bb[:, :, None], BL, axis=2))

    return lay_w(weff), lay_w(weff0), lay_b(beff), lay_b(beff0)


def _conv_layout(w):
    # [Cout, Cin, K] -> [Cin, K, Cout] -> partition-chunked [128, kt, K, Cout]
    cin = w.shape[1]
    wt = np.ascontiguousarray(w.transpose(1, 2, 0)).astype(_FP)  # [Cin, K, Cout]
    if cin > 128:
        kt = cin // 128
        return np.ascontiguousarray(
            wt.reshape(kt, 128, w.shape[2], w.shape[0]).transpose(1, 0, 2, 3))
    return wt  # [Cin<=128, K, Cout]


def _chunk_bias(b):
    # [C] -> [128, C//128]
    return np.ascontiguousarray(b.astype(_FP).reshape(-1, 128).T)


def _sq_layout(w):
    # [M, K] dense square-ish weight -> lhsT tiles [128, kt, mc, 128]
    M, Kd = w.shape
    wt = w.T.astype(_FP)  # [K, M]
    kt, mc = Kd // 128, M // 128
    return np.ascontiguousarray(
        wt.reshape(kt, 128, mc, 128).transpose(1, 0, 2, 3))


def _prep_consts(d):
    c = {}
    # --- TCN ---
    c["l0w1T"] = _conv_layout(d["l0_w1"])            # [16, 4, 256]
    c["l0dwT"] = _conv_layout(d["l0_dw"])            # [16, 1, 256]
    c["l0w2T"] = _conv_layout(d["l0_w2"])            # [128, 2, 4, 256]
    c["l1w1T"] = _conv_layout(d["l1_w1"])
    c["l1w2T"] = _conv_layout(d["l1_w2"])
    c["l2w1T"] = _conv_layout(d["l2_w1"])
    c["l2w2T"] = _conv_layout(d["l2_w2"])
    for nm in ["l0_b1", "l0_b2", "l0_db", "l1_b1", "l1_b2", "l2_b1", "l2_b2"]:
        c[nm.replace("_", "")] = _chunk_bias(d[nm])  # [128, 2]
    # --- LSTMs ---
    e = _lstm_layout(d["e_wih"], d["e_whh"], d["e_bih"], d["e_bhh"],
                     d["e_fcw"], d["e_fcb"], d["h2i_w"], d["h2i_b"])
    c["ewhT"], c["ewh0T"], c["ebias"], c["ebias0"] = e
    f = _lstm_layout(d["f_wih"], d["f_whh"], d["f_bih"], d["f_bhh"],
                     d["f_fcw"], d["f_fcb"], d["ar_w"][0:1, :], d["ar_b"][0:1])
    c["fwhT"], c["fwh0T"], c["fbias"], c["fbias0"] = f
    c["efcw"] = np.ascontiguousarray(d["e_fcw"].astype(_FP).reshape(2, 128).T)  # [128,2]
    c["ffcw"] = np.ascontiguousarray(d["f_fcw"].astype(_FP).reshape(2, 128).T)
    c["efcb"] = d["e_fcb"].astype(_FP).reshape(1, 1)
    c["ffcb"] = d["f_fcb"].astype(_FP).reshape(1, 1)
    # --- heads ---
    c["arwT"] = np.ascontiguousarray(
        d["ar_w"].T.astype(_FP).reshape(2, 128, H).transpose(1, 0, 2))  # [128,2,96]
    c["arb"] = d["ar_b"].astype(_FP).reshape(H, 1)
    c["dw1T"] = _sq_layout(d["dag_w1"])              # [128, 4, 4, 128]
    c["dw2T"] = _sq_layout(d["dag_w2"])
    c["db1"] = np.ascontiguousarray(d["dag_b1"].astype(_FP).reshape(4, 128).T)  # [128,4]
    c["db2"] = np.ascontiguousarray(d["dag_b2"].astype(_FP).reshape(4, 128).T)
    c["gwT"] = np.ascontiguousarray(
        d["gds_w"].T.astype(_FP).reshape(4, 128, H).transpose(1, 0, 2))  # [128,4,96]
    c["gb"] = d["gds_b"].astype(_FP).reshape(H, 1)
    return c


# ---------------------------------------------------------------------------
# Device program.

def _build_nc(const_shapes, steps_e=T, steps_f=H):
    import concourse.bass as bass
    import concourse.tile as tile
    from concourse import mybir

    _install_tile_drain_fix()
    dt = mybir.dt.float32
    AF = mybir.ActivationFunctionType
    ALU = mybir.AluOpType

    nc = bass.Bass("TRN2", debug=False)

    # per-core inputs, host-laid-out: window [F, BL, WIN], input_main [128, 4, BL]
    xwin = nc.dram_tensor("xwin", [F, BL, WIN], dt, kind="ExternalInput")
    xim = nc.dram_tensor("xim", [128, T // 128, BL], dt, kind="ExternalInput")
    cst = {}
    for name, shp in const_shapes.items():
        cst[name] = nc.dram_tensor(name, list(shp), dt, kind="ExternalInput")

    o_est = nc.dram_tensor("est", [T, BL], dt, kind="ExternalOutput")
    o_nxt = nc.dram_tensor("nxt", [T, BL], dt, kind="ExternalOutput")
    o_fc = nc.dram_tensor("fcst", [steps_f, BL], dt, kind="ExternalOutput")
    o_ar = nc.dram_tensor("ar", [H, BL], dt, kind="ExternalOutput")
    o_gds = nc.dram_tensor("gds", [H, BL], dt, kind="ExternalOutput")

    with tile.TileContext(nc) as tc:
        with tc.tile_pool(name="w", bufs=1) as wp, \
             tc.tile_pool(name="state", bufs=1) as st:
            # ---- persistent weights on SBUF ----
            w_sb = {}
            for name in const_shapes:
                t_ = wp.tile(list(const_shapes[name]), dt, tag=name, name="w_" + name)
                nc.gpsimd.dma_start(out=t_, in_=cst[name].ap())
                w_sb[name] = t_

            # ---- big state buffers ----
            e_h = st.tile([128, steps_e + 1, 2, BL], dt, tag="e_h")
            f_h = st.tile([128, steps_f + 1, 2, BL], dt, tag="f_h")
            ey = st.tile([1, steps_e, BL], dt, tag="ey")
            fy = st.tile([1, steps_f, BL], dt, tag="fy")
            e_tgc = [st.tile([128, 4, BL], dt, tag=f"e_tgc{i}", name=f"e_tgc{i}")
                     for i in range(2)]
            f_tgc = [st.tile([128, 4, BL], dt, tag=f"f_tgc{i}", name=f"f_tgc{i}")
                     for i in range(2)]
            nc.vector.memset(e_tgc[0][:, 2:4, :], 0.0)
            nc.vector.memset(f_tgc[0][:, 2:4, :], 0.0)

            # ================= TCN =================
            with tc.tile_pool(name="tps", bufs=2, space="PSUM") as tpp, \
                 tc.tile_pool(name="tsb", bufs=2) as tsb:
                xw = st.tile([16, BL, WF], dt, tag="xw")
                nc.vector.memset(xw[:, :, 0:PAD], 0.0)
                nc.gpsimd.dma_start(out=xw[:, :, PAD:WF], in_=xwin.ap())

                def conv_into(psum, w_t, cin_tiles, taps, dil, rhs_of, mc,
                              extra=None):
                    first = True
                    for kt in range(cin_tiles):
                        for tap in range(taps):
                            lhsT = (w_t[:, tap, mc * 128:(mc + 1) * 128]
                                    if cin_tiles == 1 else
                                    w_t[:, kt, tap, mc * 128:(mc + 1) * 128])
                            rhs = rhs_of(kt, tap, dil)
                            last = (kt == cin_tiles - 1 and tap == taps - 1
                                    and extra is None)
                            nc.tensor.matmul(psum, lhsT, rhs,
                                             start=first, stop=last)
                            first = False
                    if extra is not None:
                        extra(psum)

                def block(in_t, w1, b1, w2, b2, cin_tiles, dil, out_t,
                          dw=None, db=None):
                    # conv1 + relu
                    o1 = tsb.tile([128, 2, BL, WF], dt, tag="o1")
                    nc.vector.memset(o1[:, :, :, 0:PAD], 0.0)
                    for mc in range(2):
                        ps = tpp.tile([128, BL, WIN], dt, tag="cps")
                        conv_into(
                            ps, w1, cin_tiles, K, dil,
                            lambda kt, tap, d: (
                                in_t[:, :, PAD - (K - 1 - tap) * d:
                                     PAD - (K - 1 - tap) * d + WIN]
                                if cin_tiles == 1 else
                                in_t[:, kt, :, PAD - (K - 1 - tap) * d:
                                     PAD - (K - 1 - tap) * d + WIN]),
                            mc)
                        nc.scalar.activation(
                            out=o1[:, mc, :, PAD:WF], in_=ps, func=AF.Relu,
                            bias=b1[:, mc:mc + 1], scale=1.0)
                    # conv2 (+ residual) + relu
                    for mc in range(2):
                        ps = tpp.tile([128, BL, WIN], dt, tag="cps")
                        conv_into(
                            ps, w2, 2, K, dil,
                            lambda kt, tap, d: o1[:, kt, :,
                                                  PAD - (K - 1 - tap) * d:
                                                  PAD - (K - 1 - tap) * d + WIN],
                            mc)
                        s1 = tsb.tile([128, BL, WIN], dt, tag="s1")
                        if dw is not None:
                            # residual = dw @ x + db (1x1 conv from input window)
                            ps2 = tpp.tile([128, BL, WIN], dt, tag="cps2")
                            nc.tensor.matmul(
                                ps2, dw[:, 0, mc * 128:(mc + 1) * 128],
                                in_t[:, :, PAD:WF], start=True, stop=True)
                            # s1 = relu(ps + b2) ; out = relu(s1 + ps2 + db)
                            nc.scalar.activation(out=s1, in_=ps, func=AF.Relu,
                                                 bias=b2[:, mc:mc + 1], scale=1.0)
                            s2 = tsb.tile([128, BL, WIN], dt, tag="s2")
                            nc.vector.scalar_tensor_tensor(
                                out=s2, in0=ps2, scalar=db[:, mc:mc + 1],
                                in1=s1, op0=ALU.add, op1=ALU.add)
                            nc.scalar.activation(out=out_t[:, mc, :, PAD:WF],
                                                 in_=s2, func=AF.Relu)
                        else:
                            nc.scalar.activation(out=s1, in_=ps, func=AF.Relu,
                                                 bias=b2[:, mc:mc + 1], scale=1.0)
                            s2 = tsb.tile([128, BL, WIN], dt, tag="s2")
                            nc.vector.tensor_add(
                                s2, s1, in_t[:, mc, :, PAD:WF])
                            nc.scalar.activation(out=out_t[:, mc, :, PAD:WF],
                                                 in_=s2, func=AF.Relu)

                t0 = tsb.tile([128, 2, BL, WF], dt, tag="t0")
                nc.vector.memset(t0[:, :, :, 0:PAD], 0.0)
                block(xw, w_sb["l0w1T"], w_sb["l0b1"], w_sb["l0w2T"],
                      w_sb["l0b2"], 1, 1, t0, dw=w_sb["l0dwT"], db=w_sb["l0db"])
                t1 = tsb.tile([128, 2, BL, WF], dt, tag="t1")
                nc.vector.memset(t1[:, :, :, 0:PAD], 0.0)
                block(t0, w_sb["l1w1T"], w_sb["l1b1"], w_sb["l1w2T"],
                      w_sb["l1b2"], 2, 2, t1)
                t2 = tsb.tile([128, 2, BL, WF], dt, tag="t2")
                block(t1, w_sb["l2w1T"], w_sb["l2b1"], w_sb["l2w2T"],
                      w_sb["l2b2"], 2, 4, t2)

                # hidden = t2[:, :, :, PAD+WIN-1] -> h0 for both LSTMs
                for mc in range(2):
                    hid = t2[:, mc, :, WF - 1:WF]  # [128, BL, 1]
                    nc.vector.tensor_copy(e_h[:, 0, mc, :], hid)
                    nc.vector.tensor_copy(f_h[:, 0, mc, :], hid)

            # ================= AR head =================
            with tc.tile_pool(name="hps", bufs=2, space="PSUM") as hp:
                ps = hp.tile([H, BL], dt, tag="arps")
                for kt in range(2):
                    nc.tensor.matmul(ps, w_sb["arwT"][:, kt, :],
                                     e_h[:, 0, kt, :],
                                     start=(kt == 0), stop=(kt == 1))
                ar_sb = st.tile([H, BL], dt, tag="ar_sb")
                nc.vector.tensor_scalar_add(out=ar_sb, in0=ps,
                                            scalar1=w_sb["arb"])
                nc.gpsimd.dma_start(out=o_ar.ap(), in_=ar_sb)

            # ================= LSTM decoders =================
            with tc.tile_pool(name="eg", bufs=2, space="PSUM") as egp, \
                 tc.tile_pool(name="fg", bufs=2, space="PSUM") as fgp, \
                 tc.tile_pool(name="lsb", bufs=3) as lsb:

                def lstm_round(t, which):
                    if which == "e":
                        h_all, tgc, gp = e_h, e_tgc, egp
                        wT = w_sb["ewh0T"] if t == 0 else w_sb["ewhT"]
                        bias = w_sb["ebias0"] if t == 0 else w_sb["ebias"]
                    else:
                        h_all, tgc, gp = f_h, f_tgc, fgp
                        wT = w_sb["fwh0T"] if t == 0 else w_sb["fwhT"]
                        bias = w_sb["fbias0"] if t == 0 else w_sb["fbias"]
                    ps = gp.tile([128, 8, BL], dt, tag=which + "g")
                    for j in range(8):
                        for kt in range(2):
                            nc.tensor.matmul(
                                ps[:, j, :],
                                wT[:, kt, j * 128:(j + 1) * 128],
                                h_all[:, t, kt, :],
                                start=(kt == 0), stop=(kt == 1))
                    g_sb = lsb.tile([128, 8, BL], dt, tag=which + "gs")
                    nc.vector.tensor_add(g_sb, ps, bias)
                    sig = lsb.tile([128, 6, BL], dt, tag=which + "sig")
                    nc.scalar.activation(out=sig, in_=g_sb[:, 0:6, :],
                                         func=AF.Sigmoid)
                    cur, nxt_ = tgc[t % 2], tgc[(t + 1) % 2]
                    nc.scalar.activation(out=cur[:, 0:2, :],
                                         in_=g_sb[:, 6:8, :], func=AF.Tanh)
                    prod = lsb.tile([128, 4, BL], dt, tag=which + "pr")
                    nc.vector.tensor_mul(prod, sig[:, 0:4, :], cur)
                    nc.vector.tensor_add(nxt_[:, 2:4, :], prod[:, 0:2, :],
                                         prod[:, 2:4, :])
                    tcs = lsb.tile([128, 2, BL], dt, tag=which + "tc")
                    nc.scalar.activation(out=tcs, in_=nxt_[:, 2:4, :],
                                         func=AF.Tanh)
                    nc.vector.tensor_mul(h_all[:, t + 1, :, :],
                                         sig[:, 4:6, :], tcs)

                for t in range(steps_e):
                    lstm_round(t, "e")
                    if t < steps_f:
                        lstm_round(t, "f")
                for t in range(steps_e, steps_f):  # if steps_f > steps_e
                    lstm_round(t, "f")

            # ================= y extraction =================
            with tc.tile_pool(name="yps", bufs=2, space="PSUM") as yp:
                def extract_y(h_all, fcw, fcb, y_sb, steps):
                    done = 0
                    while done < steps:
                        n = min(64, steps - done)
                        ps = yp.tile([1, 64 * BL], dt, tag="yps")
                        for kt in range(2):
                            rhs = h_all[:, 1 + done:1 + done + n, kt, :]
                            nc.tensor.matmul(ps[:, 0:n * BL], fcw[:, kt:kt + 1],
                                             rhs, start=(kt == 0), stop=(kt == 1))
                        nc.vector.tensor_scalar_add(
                            out=y_sb[:, done:done + n, :],
                            in0=ps[:, 0:n * BL].rearrange("p (t b) -> p t b", b=BL),
                            scalar1=fcb)
                        done += n

                extract_y(e_h, w_sb["efcw"], w_sb["efcb"], ey, steps_e)
                extract_y(f_h, w_sb["ffcw"], w_sb["ffcb"], fy, steps_f)
                nc.gpsimd.dma_start(out=o_est.ap()[0:steps_e, :],
                                    in_=ey[0:1, :, :])
                nc.gpsimd.dma_start(out=o_fc.ap(), in_=fy[0:1, :, :])

            # ================= gating MLP =================
            if steps_e != T:
                return nc  # reduced-step debug build: skip gating
            with tc.tile_pool(name="gps", bufs=2, space="PSUM") as gp2, \
                 tc.tile_pool(name="gsb", bufs=2) as gsb:
                # yT / imT in [t0(part), t1, b] layout
                yT = gsb.tile([128, 4, BL], dt, tag="yT")
                # partition-scatter reshape; DMA APs cap at 3 dims -> 4 DMAs
                for t1 in range(4):
                    nc.gpsimd.dma_start(
                        out=yT[:, t1, :],
                        in_=ey[0:1, t1 * 128:(t1 + 1) * 128, :])
                imT = gsb.tile([128, 4, BL], dt, tag="imT")
                nc.gpsimd.dma_start(out=imT, in_=xim.ap())
                aT = gsb.tile([128, 4, BL], dt, tag="aT")
                nc.vector.tensor_sub(aT, imT, yT)
                h1T = gsb.tile([128, 4, BL], dt, tag="h1T")
                for mc in range(4):
                    ps = gp2.tile([128, BL], dt, tag="mps")
                    for kt in range(4):
                        nc.tensor.matmul(ps, w_sb["dw1T"][:, kt, mc, :],
                                         aT[:, kt, :],
                                         start=(kt == 0), stop=(kt == 3))
                    nc.scalar.activation(out=h1T[:, mc, :], in_=ps,
                                         func=AF.Relu,
                                         bias=w_sb["db1"][:, mc:mc + 1],
                                         scale=1.0)
                alT = gsb.tile([128, 4, BL], dt, tag="alT")
                for mc in range(4):
                    ps = gp2.tile([128, BL], dt, tag="mps")
                    for kt in range(4):
                        nc.tensor.matmul(ps, w_sb["dw2T"][:, kt, mc, :],
                                         h1T[:, kt, :],
                                         start=(kt == 0), stop=(kt == 3))
                    nc.scalar.activation(out=alT[:, mc, :], in_=ps,
                                         func=AF.Sigmoid,
                                         bias=w_sb["db2"][:, mc:mc + 1],
                                         scale=1.0)
                goT = gsb.tile([128, 4, BL], dt, tag="goT")
                nc.vector.tensor_mul(goT, alT, aT)
                nxT = gsb.tile([128, 4, BL], dt, tag="nxT")
                nc.vector.tensor_sub(nxT, imT, goT)
                nc.gpsimd.dma_start(
                    out=o_nxt.ap().rearrange("(t1 t0) b -> t0 t1 b", t0=128),
                    in_=nxT)
                # gds_horizon
                ps = gp2.tile([H, BL], dt, tag="gh")
                for kt in range(4):
                    nc.tensor.matmul(ps, w_sb["gwT"][:, kt, :], goT[:, kt, :],
                                     start=(kt == 0), stop=(kt == 3))
                gh_sb = gsb.tile([H, BL], dt, tag="gh_sb")
                nc.vector.tensor_scalar_add(out=gh_sb, in0=ps,
                                            scalar1=w_sb["gb"])
                nc.gpsimd.dma_start(out=o_gds.ap(), in_=gh_sb)

    return nc


# ---------------------------------------------------------------------------

_BUILD_CACHE = {}
TRACE = False        # set True to capture an NTFF profile (fills LAST_RESULT)
LAST_RESULT = None


def _get_nc(const_shapes, steps_e, steps_f):
    key = (tuple(sorted((k, v) for k, v in const_shapes.items())),
           steps_e, steps_f)
    if key not in _BUILD_CACHE:
        _BUILD_CACHE[key] = _build_nc(dict(const_shapes), steps_e, steps_f)
    return _BUILD_CACHE[key]


def kernel(**inputs):
    from concourse import bass_utils

    d = {k: np.asarray(v, dtype=np.float32) for k, v in inputs.items()}
    consts = _prep_consts(d)
    const_shapes = {k: tuple(v.shape) for k, v in consts.items()}
    nc = _get_nc(const_shapes, T, H)

    x = d["inputs"]
    in_maps = []
    for c in range(NCORES):
        sl = x[c * BL:(c + 1) * BL]
        m = dict(consts)
        m["xwin"] = np.ascontiguousarray(sl[:, T - WIN:, :].transpose(2, 0, 1))
        m["xim"] = np.ascontiguousarray(
            sl[:, :, 0].T.reshape(T // 128, 128, BL).transpose(1, 0, 2))
        in_maps.append(m)

    res = bass_utils.run_bass_kernel_spmd(nc, in_maps,
                                          core_ids=list(range(NCORES)),
                                          trace=TRACE)
    global LAST_RESULT
    LAST_RESULT = res

    est = np.concatenate([r["est"].T for r in res.results], axis=0)  # [B, T]
    nxt = np.concatenate([r["nxt"].T for r in res.results], axis=0)
    fc = np.concatenate([r["fcst"].T for r in res.results], axis=0)  # [B, H]
    ar = np.concatenate([r["ar"].T for r in res.results], axis=0)
    gds = np.concatenate([r["gds"].T for r in res.results], axis=0)

    input_main = np.ascontiguousarray(x[:, :, 0:1])
    return (fc[:, :, None], nxt[:, :, None], ar[:, :, None],
            est[:, :, None], input_main, gds[:, :, None])
